# revision 27
# baseline (speedup 1.0000x reference)
"""Trainium2 Bass kernel for nn_Block_50113678410401 (dense transformer block).

Strategy: data-parallel over the batch axis (B=8 -> 8 NeuronCores, one batch
element per core). All on-chip activations live in "layout A": feature axis on
SBUF partitions, token axis (T) on the free dimension, so no on-chip
transposes are needed (host pre-transposes x and post-transposes the output).

v2: the q/k/v and output projections run in fp8 e4m3 with DoubleRow perf
mode - two K=128 contraction tiles per matmul instruction, ~2x tensor-
engine throughput. Weights are pre-scaled on the host into fp8 range
(scales folded back out via the activation-engine `scale` operand on the
PSUM->SBUF path). The attention core (scores, softmax, attn@V) stays bf16
(the no-max-sub exp overflows fp8's range), and the FFN stays bf16: fp8
there costs ~3.5% relative error against the 2% tolerance (measured in a
host simulation), while the attention-side fp8 paths cost ~1.5% combined.

Per core:
  LN1 (stats via ones-matmul over partitions), per-head causal attention
  (no-max-sub exp softmax, denominator via ones-matmul, normalization via
  K=1 broadcast matmul + wide reciprocal), output projection + residual
  (+bo folded into the host-prepared residual copy of x), BatchNorm over
  (B,C) with per-512-chunk cross-core AllReduces of (sum, sumsq) per T
  channel, LN2+FFN, residual, second BatchNorm.

LN2-of-BN1 shortcut: BN1 is a per-T-channel affine, and LayerNorm is
invariant to per-token affine maps up to the eps term: LN2(BN1(u)) =
(u - mean_c u) / sqrt(var_c u + eps/s^2) where s is BN1's global scale.
Substituting eps/s^2 -> eps changes the result by O(1e-5) relative, so the
FFN input is computed purely from LOCAL per-token stats and never waits on
the BN1 AllReduce. Only the residual branch (u1 -> BN1(u1), needed before
the FFN output is added) consumes the AllReduce, off the critical path.

All statistics, softmax, residuals and normalizations are fp32.
LayerNorm/projection affine parameters are folded into the weights on the
host: wq' = diag(ln1_g) wq / sqrt(D) (q also carries 1/sqrt(D)), k-side
bias drops out of softmax by shift invariance, v-side bias is folded into
the output-projection bias (which itself is folded into the residual x
copy), ln2 affine is folded into w1/b1.
"""

import numpy as np
import ml_dtypes

B, T, C, H, D = 8, 1024, 1536, 12, 128
F = 4 * C            # 6144
P = 128
CT = C // P          # 12 c-tiles
KP = CT // 2         # 6 c-tile pairs (DoubleRow)
FT = F // P          # 48 f-tiles
FP = FT // 2         # 24 f-tile pairs
HP = H // 2          # 6 head pairs
ST = T // P          # 8 s-tiles
CH = 512             # matmul free-dim chunk
NCH = T // CH        # 2 chunks
EPS = 1e-5
NCORES = 8
NBC = B * C          # BatchNorm count over (B, C)

# fp8 weight pre-scales (folded out on the PSUM->SBUF activation path)
SQ, SK, SV, SO, S1, S2 = 512.0, 64.0, 64.0, 64.0, 64.0, 64.0

_PROG = None


def _build():
    import concourse.bass as bass
    import concourse.mybir as mybir
    import concourse.tile as tile
    from concourse import bacc
    from concourse.masks import make_upper_triangular

    fp32 = mybir.dt.float32
    bf16 = mybir.dt.bfloat16
    fp8 = mybir.dt.float8e4
    AF = mybir.ActivationFunctionType
    OP = mybir.AluOpType
    DR = mybir.MatmulPerfMode.DoubleRow
    ts = bass.ts

    nc = bacc.Bacc("TRN2", target_bir_lowering=False, debug=False,
                   enable_asserts=True, num_devices=NCORES)

    # ---- DRAM I/O (weights host-pretiled for contiguous DMA) ----
    xT_d = nc.dram_tensor("xT", (CT, P, T), fp32, kind="ExternalInput").ap()
    xbf_d = nc.dram_tensor("xbf", (CT, P, T), bf16, kind="ExternalInput").ap()
    wq_d = nc.dram_tensor("wq", (H, P, CT, P), fp8, kind="ExternalInput").ap()
    wk_d = nc.dram_tensor("wk", (H, P, CT, P), fp8, kind="ExternalInput").ap()
    wv_d = nc.dram_tensor("wv", (C, C), bf16, kind="ExternalInput").ap()
    bq_d = nc.dram_tensor("bq", (P, H), fp32, kind="ExternalInput").ap()
    wo_d = nc.dram_tensor("wo", (CT, P, H, P), bf16, kind="ExternalInput").ap()
    w1_d = nc.dram_tensor("w1", (FT, P, CT, P), bf16, kind="ExternalInput").ap()
    b1_d = nc.dram_tensor("b1", (P, FT), fp32, kind="ExternalInput").ap()
    w2_d = nc.dram_tensor("w2", (CT, P, FT, P), bf16, kind="ExternalInput").ap()
    b2_d = nc.dram_tensor("b2", (P, CT), fp32, kind="ExternalInput").ap()
    bn1g_d = nc.dram_tensor("bn1g", (1, T), fp32, kind="ExternalInput").ap()
    bn1b_d = nc.dram_tensor("bn1b", (1, T), fp32, kind="ExternalInput").ap()
    bn2g_d = nc.dram_tensor("bn2g", (1, T), fp32, kind="ExternalInput").ap()
    bn2b_d = nc.dram_tensor("bn2b", (1, T), fp32, kind="ExternalInput").ap()
    yT_d = nc.dram_tensor("yT", (CT, P, T), fp32, kind="ExternalOutput").ap()

    with tile.TileContext(nc) as tc:
        with tc.tile_pool(name="const", bufs=1) as cpool, \
             tc.tile_pool(name="scratch", bufs=1) as spool, \
             tc.tile_pool(name="u1p", bufs=1) as u1pool, \
             tc.tile_pool(name="wstr", bufs=1) as wstream, \
             tc.tile_pool(name="ppw", bufs=6, space="PSUM") as ppw, \
             tc.tile_pool(name="pps", bufs=2, space="PSUM") as pps, \
             tc.tile_pool(name="dram", bufs=1, space="DRAM") as dpool:

            # ---- persistent residual tiles: u1 = x (+bo2, host-folded) ----
            # (DMAs issued after the LN1 chunk-0 affines so they don't
            # steal HBM bandwidth from the xbf loads LN1 stats wait on)
            u1 = [u1pool.tile([P, T], fp32, tag=f"u{k}", name=f"u1_{k}")
                  for k in range(CT)]

            # ---- constants ----
            ones_bf = cpool.tile([P, 1], bf16, name="ones_bf")
            nc.vector.memset(ones_bf[:], 1.0)
            trimask = cpool.tile([P, P], bf16, name="trimask")
            make_upper_triangular(nc, trimask[:], val=1.0, diag=True)
            bq_sb = cpool.tile([P, H], fp32, name="bq_sb")
            nc.sync.dma_start(bq_sb[:], bq_d[:])
            b1_sb = cpool.tile([P, FT], fp32, name="b1_sb")
            nc.sync.dma_start(b1_sb[:], b1_d[:])
            b2_sb = cpool.tile([P, CT], fp32, name="b2_sb")
            nc.sync.dma_start(b2_sb[:], b2_d[:])

            # PE clock warm-up: ~5us of junk matmuls during the initial
            # x DMA so the LN1 stats matmuls run at 2.4 GHz, not 1.2.
            warm_ps = ppw.tile([P, P], fp32, tag="w", name="warm_ps")
            for _ in range(40):
                nc.tensor.matmul(warm_ps[:], trimask[:], trimask[:],
                                 start=True, stop=True)

            # ---- helpers ----
            def bcast_into(dst_ap, row_ap, name, n=CH, eng=None):
                """(1, n) fp32 SBUF row -> (P, n) via DRAM bounce on the given
                DMA-capable engine queue (default scalar)."""
                e = eng if eng is not None else nc.scalar
                dr = dpool.tile([1, n], fp32, tag="bcd", bufs=4,
                                name=f"{name}_dr")
                e.dma_start(dr[:], row_ap)
                e.dma_start(dst_ap, dr[:].to_broadcast((P, n)))

            ones1f = cpool.tile([1, P], fp32, name="ones1f")
            nc.vector.memset(ones1f[:], 1.0)

            def bc_mm_into(dst_ap, row_ap, name):
                """(1, CH) fp32 row -> (P, CH) SBUF via K=1 matmul + copy.
                Higher PE cost than bcast_into but ~3x lower latency; used on
                norm-param critical paths where the PE is idle anyway."""
                ps = ppw.tile([P, CH], fp32, tag="w", name=f"{name}_ps")
                nc.tensor.matmul(ps[:], ones1f[:], row_ap, start=True,
                                 stop=True)
                nc.scalar.copy(dst_ap, ps[:])

            # Packed stat psum tile: row 0 accumulates sum, row 32 sumsq.
            def stat_tiles(name):
                return [pps.tile([P, CH], fp32, tag="st", bufs=2,
                                 name=f"{name}_{j}") for j in range(NCH)]

            def stats_chunk(src_ap, stp_j, first, last, is_bf16=False):
                """Ones-matmul partial sums of src chunk ((P,CH)) and its
                square into packed stat rows."""
                if is_bf16:
                    cbf = src_ap
                else:
                    cbf_t = spool.tile([P, CH], bf16, tag="cast_bf", bufs=2,
                                       name="cbf")
                    nc.vector.tensor_copy(cbf_t[:], src_ap)
                    cbf = cbf_t[:]
                csq = spool.tile([P, CH], bf16, tag="cast_sq", bufs=2,
                                 name="csq")
                nc.scalar.square(csq[:], src_ap)
                nc.tensor.matmul(stp_j[0:1, :], ones_bf[:], cbf,
                                 start=first, stop=last)
                nc.tensor.matmul(stp_j[32:33, :], ones_bf[:], csq[:],
                                 start=first, stop=last)

            def allreduce_chunk(pool, stp_j, name):
                """AllReduce-add this chunk's packed (sum, sumsq) across
                cores. Returns the (local, global) rows."""
                loc = pool.tile([1, 2 * CH], fp32, tag="arloc", bufs=2,
                                name=f"{name}_loc")
                nc.scalar.copy(loc[:, 0:CH], stp_j[0:1, :])
                nc.scalar.copy(loc[:, CH:2 * CH], stp_j[32:33, :])
                cin = dpool.tile([1, 2 * CH], fp32, name=f"{name}_cin")
                cout = dpool.tile([1, 2 * CH], fp32, name=f"{name}_cout")
                nc.gpsimd.dma_start(cin[:], loc[:])
                nc.gpsimd.collective_compute(
                    "AllReduce", mybir.AluOpType.add,
                    replica_groups=[list(range(NCORES))],
                    ins=[cin.opt()], outs=[cout.opt()],
                )
                glob = pool.tile([1, 2 * CH], fp32, tag="arglob", bufs=2,
                                 name=f"{name}_glob")
                nc.gpsimd.dma_start(glob[:], cout[:])
                return loc, glob

            def norm_params_chunk(pool, s1_ap, s2_ap, count, name,
                                  g_row_sl=None, b_row_sl=None,
                                  sc_tag="nsc", bi_tag="nbi", bc_pool=None,
                                  bc_bufs=1, via_dma=False):
                """Per-chunk normalization params, computed at row level
                (single-lane, cheap custom-DVE reciprocal), then broadcast
                to (P, CH). Returns (sc_bc, bi_bc)."""
                m = pool.tile([1, CH], fp32, tag="rm", bufs=1,
                              name=f"{name}_m")
                nc.vector.tensor_scalar_mul(m[:], s1_ap, 1.0 / count)
                v = pool.tile([1, CH], fp32, tag="rv", bufs=1,
                              name=f"{name}_v")
                nc.vector.tensor_scalar_mul(v[:], s2_ap, 1.0 / count)
                bias = pool.tile([1, CH], fp32, tag="rb", bufs=1,
                                 name=f"{name}_brow")
                nc.vector.tensor_mul(bias[:], m[:], m[:])
                nc.vector.tensor_sub(v[:], v[:], bias[:])
                nc.vector.tensor_scalar_add(v[:], v[:], EPS)
                nc.scalar.sqrt(v[:], v[:])
                scale = pool.tile([1, CH], fp32, tag="rs", bufs=1,
                                  name=f"{name}_srow")
                if g_row_sl is not None:
                    rc = pool.tile([1, CH], fp32, tag="rr", bufs=1,
                                   name=f"{name}_rc")
                    nc.vector.reciprocal_approx_fast(rc[:], v[:])
                    nc.vector.tensor_mul(scale[:], rc[:], g_row_sl)
                else:
                    nc.vector.reciprocal_approx_fast(scale[:], v[:])
                nc.vector.tensor_mul(bias[:], m[:], scale[:])
                nc.vector.tensor_scalar_mul(bias[:], bias[:], -1.0)
                if b_row_sl is not None:
                    nc.vector.tensor_add(bias[:], bias[:], b_row_sl)
                bpool = bc_pool if bc_pool is not None else pool
                sc_bc = bpool.tile([P, CH], fp32, tag=sc_tag, bufs=bc_bufs,
                                   name=f"{name}_scbc")
                bi_bc = bpool.tile([P, CH], fp32, tag=bi_tag, bufs=bc_bufs,
                                   name=f"{name}_bibc")
                if via_dma:
                    bcast_into(sc_bc[:], scale[:], f"{name}_sc",
                               eng=nc.gpsimd)
                    bcast_into(bi_bc[:], bias[:], f"{name}_bi",
                               eng=nc.gpsimd)
                else:
                    bc_mm_into(sc_bc[:], scale[:], f"{name}_sc")
                    bc_mm_into(bi_bc[:], bias[:], f"{name}_bi")
                return sc_bc, bi_bc

            def affine_chunk(dst_ap, src_ap, sc_ap, bi_ap, eng=None):
                """dst = src * sc + bi on one (P, CH) chunk."""
                e = eng if eng is not None else nc.vector
                tag = "ntmpg" if e is nc.gpsimd else "ntmp"
                tmp = spool.tile([P, CH], fp32, tag=tag, bufs=1,
                                 name="ntmp")
                e.tensor_mul(tmp[:], src_ap, sc_ap)
                e.tensor_add(dst_ap, tmp[:], bi_ap)

            with tc.tile_pool(name="onrm", bufs=1) as opool:
                o_nrm = opool.tile([P, H, T], bf16, name="o_nrm")
                with tc.tile_pool(name="hT", bufs=1) as hpool:
                    hT = hpool.tile([P, CT, T], fp8, name="hT")
                    hTb = [hpool.tile([P, T], bf16, tag=f"hb{k}",
                                      name=f"hTb_{k}") for k in range(CT)]
                    # ================= Phase 1: LN1 =================
                    with tc.tile_pool(name="vall2", bufs=1) as vpool:
                        Vall = [vpool.tile([P, C], bf16, tag=f"v{s}",
                                           name=f"V_{s}") for s in range(ST)]
                        with tc.tile_pool(name="p1", bufs=1) as p1:
                            stp = stat_tiles("ln1")
                            for k in range(CT):
                                nc.sync.dma_start(hTb[k][:], xbf_d[k])
                                for j in range(NCH):
                                    sl = slice(j * CH, (j + 1) * CH)
                                    stats_chunk(hTb[k][:, sl], stp[j], k == 0,
                                                k == CT - 1, is_bf16=True)
                            ln1p = []
                            for j in range(NCH):
                                ln1p.append(norm_params_chunk(
                                    p1, stp[j][0:1, :], stp[j][32:33, :], C,
                                    f"ln1_{j}", bc_bufs=1))
                            # chunk-0 affines, V for the first four s-tiles,
                            # then chunk-1 affines, V for the rest: tile-level
                            # dependency tracking makes any hT reader wait for
                            # every PRIOR hT write, so V is emitted as early
                            # as its inputs allow.
                            sc0, bi0 = ln1p[0]
                            for k in range(CT):
                                affine_chunk(hTb[k][:, 0:CH], hTb[k][:, 0:CH],
                                             sc0[:], bi0[:],
                                             eng=(nc.vector if k < 8
                                                  else nc.gpsimd))
                                nc.scalar.copy(hT[:, k, 0:CH],
                                               hTb[k][:, 0:CH])
                            for k in range(CT):
                                nc.scalar.dma_start(u1[k][:], xT_d[k])

                            with tc.tile_pool(name="wv", bufs=1) as wvpool:
                                def v_pass(s_lo, s_hi, tag):
                                    for n in range(C // CH):
                                        wv_sb = []
                                        for k in range(CT):
                                            wvk = wvpool.tile(
                                                [P, CH], bf16, tag=f"wv{k}",
                                                bufs=2,
                                                name=f"wv_{k}_{n}_{tag}")
                                            nc.sync.dma_start(
                                                wvk[:],
                                                wv_d[ts(k, P), ts(n, CH)])
                                            wv_sb.append(wvk)
                                        for s in range(s_lo, s_hi):
                                            vps = ppw.tile(
                                                [P, CH], fp32, tag="w",
                                                name=f"v_ps_{s}_{n}_{tag}")
                                            for k in range(CT):
                                                nc.tensor.matmul(
                                                    vps[:],
                                                    hTb[k][:, ts(s, P)],
                                                    wv_sb[k][:],
                                                    start=(k == 0),
                                                    stop=(k == CT - 1))
                                            nc.scalar.copy(
                                                Vall[s][:, ts(n, CH)], vps[:])

                                v_pass(0, ST // 2, "a")
                                sc1, bi1 = ln1p[1]
                                for k in range(CT):
                                    affine_chunk(hTb[k][:, CH:T],
                                                 hTb[k][:, CH:T],
                                                 sc1[:], bi1[:],
                                                 eng=(nc.vector if k < 8
                                                      else nc.gpsimd))
                                    nc.scalar.copy(hT[:, k, CH:T],
                                                   hTb[k][:, CH:T])
                                v_pass(ST // 2, ST, "b")

                        # ============ Phase 3: per-head attention ==========
                        with tc.tile_pool(name="p3", bufs=1) as p3:
                            for h in range(H):
                                wqh = p3.tile([P, CT, P], fp8, tag="wqh",
                                              bufs=2, name=f"wqh_{h}")
                                nc.sync.dma_start(wqh[:], wq_d[h])
                                wkh = p3.tile([P, CT, P], fp8, tag="wkh",
                                              bufs=2, name=f"wkh_{h}")
                                nc.sync.dma_start(wkh[:], wk_d[h])
                                qT = p3.tile([P, T], bf16, tag="qT", bufs=2,
                                             name=f"qT_{h}")
                                kT = p3.tile([P, T], bf16, tag="kT", bufs=2,
                                             name=f"kT_{h}")
                                # kp-outer, j-inner: consecutive matmuls
                                # share the same stationary weight tile
                                qps = [ppw.tile([P, CH], fp32, tag="w",
                                                name=f"q_ps_{h}_{j}")
                                       for j in range(NCH)]
                                kps = [ppw.tile([P, CH], fp32, tag="w",
                                                name=f"k_ps_{h}_{j}")
                                       for j in range(NCH)]
                                for kp in range(KP):
                                    kp_sl = slice(2 * kp, 2 * kp + 2)
                                    for j in range(NCH):
                                        sl = slice(j * CH, (j + 1) * CH)
                                        nc.tensor.matmul(qps[j][:],
                                                         wqh[:, kp_sl, :],
                                                         hT[:, kp_sl, sl],
                                                         start=(kp == 0),
                                                         stop=(kp == KP - 1),
                                                         perf_mode=DR)
                                for kp in range(KP):
                                    kp_sl = slice(2 * kp, 2 * kp + 2)
                                    for j in range(NCH):
                                        sl = slice(j * CH, (j + 1) * CH)
                                        nc.tensor.matmul(kps[j][:],
                                                         wkh[:, kp_sl, :],
                                                         hT[:, kp_sl, sl],
                                                         start=(kp == 0),
                                                         stop=(kp == KP - 1),
                                                         perf_mode=DR)
                                for j in range(NCH):
                                    sl = slice(j * CH, (j + 1) * CH)
                                    nc.scalar.activation(qT[:, sl], qps[j][:],
                                                         AF.Identity,
                                                         bias=bq_sb[:, h:h + 1],
                                                         scale=1.0 / SQ)
                                    nc.scalar.mul(kT[:, sl], kps[j][:],
                                                  1.0 / SK)
                                # scores + exp (causal: s-tile covers t >= s*P)
                                aT = []
                                for s in range(ST):
                                    at = p3.tile([P, T - s * P], bf16,
                                                 tag=f"a{s}", bufs=1,
                                                 name=f"aT_{h}_{s}")
                                    aT.append(at)
                                    for j in range(NCH):
                                        lo = max(j * CH, s * P)
                                        hi = (j + 1) * CH
                                        if lo >= hi:
                                            continue
                                        sps = ppw.tile([P, CH], fp32, tag="w",
                                                       name=f"s_ps_{h}_{s}_{j}")
                                        nc.tensor.matmul(sps[:, :hi - lo],
                                                         kT[:, ts(s, P)],
                                                         qT[:, lo:hi],
                                                         start=True, stop=True)
                                        nc.scalar.activation(
                                            at[:, lo - s * P:hi - s * P],
                                            sps[:, :hi - lo], AF.Exp)
                                    nc.vector.tensor_mul(at[:, 0:P],
                                                         at[:, 0:P],
                                                         trimask[:])
                                # denominators: packed psum, row 0 (j=0)
                                # and row 32 (j=1)
                                den_ps = pps.tile([P, CH], fp32, tag="st",
                                                  bufs=2, name=f"dn_{h}")
                                for j in range(NCH):
                                    r0 = 32 * j
                                    smax = min(ST, 4 * (j + 1))
                                    for s in range(smax):
                                        lo = max(0, s * P - j * CH)
                                        nc.tensor.matmul(
                                            den_ps[r0:r0 + 1, lo:CH],
                                            ones_bf[:],
                                            aT[s][:, j * CH + lo - s * P:
                                                  (j + 1) * CH - s * P],
                                            start=(s == 0), stop=(s == smax - 1))
                                # r_bc = 1/den broadcast: copy row, then
                                # wide reciprocal straight off the bounce
                                r_bc = p3.tile([P, T], fp32, tag="rbc", bufs=2,
                                               name=f"rbc_{h}")
                                for j in range(NCH):
                                    dj = p3.tile([1, CH], fp32, tag="den",
                                                 bufs=1, name=f"den_{h}_{j}")
                                    nc.scalar.copy(
                                        dj[:], den_ps[32 * j:32 * j + 1, :])
                                    rj = p3.tile([1, CH], fp32, tag="rrow",
                                                 bufs=1, name=f"rr_{h}_{j}")
                                    nc.vector.reciprocal_approx_fast(
                                        rj[:], dj[:])
                                    bc_mm_into(r_bc[:, j * CH:(j + 1) * CH],
                                               rj[:], f"rbc_{h}_{j}")
                                # attention @ V, then normalize
                                for j in range(NCH):
                                    smax = min(ST, 4 * (j + 1))
                                    ops_ = ppw.tile([P, CH], fp32, tag="w",
                                                    name=f"o_ps_{h}_{j}")
                                    for s in range(smax):
                                        lo = max(0, s * P - j * CH)
                                        nc.tensor.matmul(
                                            ops_[:, lo:CH],
                                            Vall[s][:, ts(h, P)],
                                            aT[s][:, j * CH + lo - s * P:
                                                  (j + 1) * CH - s * P],
                                            start=(s == 0), stop=(s == smax - 1))
                                    sl = slice(j * CH, (j + 1) * CH)
                                    nc.vector.tensor_mul(o_nrm[:, h, sl],
                                                         ops_[:], r_bc[:, sl])

                # hT/Vall closed; Phase 4: out-proj + residual + BN1 stats
                # (j-outer so chunk 0's AllReduce overlaps chunk 1's matmuls)
                stp_bn1 = stat_tiles("bn1")
                bn1_io = [None, None]
                for j in range(NCH):
                    sl = slice(j * CH, (j + 1) * CH)
                    for k in range(CT):
                        wok = opool.tile([P, H, P], bf16, tag="wok",
                                         bufs=2, name=f"wok_{j}_{k}")
                        nc.sync.dma_start(wok[:], wo_d[k])
                        saps = ppw.tile([P, CH], fp32, tag="w",
                                        name=f"sa_ps_{k}_{j}")
                        for hh in range(H):
                            nc.tensor.matmul(saps[:],
                                             wok[:, hh, :],
                                             o_nrm[:, hh, sl],
                                             start=(hh == 0),
                                             stop=(hh == H - 1))
                        # u1 = saps + x  (bo2 folded into x on host)
                        nc.vector.tensor_add(u1[k][:, sl], saps[:],
                                             u1[k][:, sl])
                        stats_chunk(u1[k][:, sl], stp_bn1[j],
                                    k == 0, k == CT - 1)
                    bn1_io[j] = allreduce_chunk(u1pool, stp_bn1[j],
                                                f"bn1_{j}")

            # ========== Phases 5-7: LN2+FFN+BN1/BN2 residual path ==========
            # h2T = LN2(BN1(u1)) from LOCAL stats only (see module docstring)
            # so the FFN starts immediately; the BN1 AllReduce result is only
            # needed for the in-place u1 affine, applied on DVE slack before
            # the FFN output lands.
            with tc.tile_pool(name="h2T", bufs=1) as h2pool:
                h2T = h2pool.tile([P, CT, T], bf16, name="h2T")
                stp_bn2 = stat_tiles("bn2")
                bn2_io = [None, None]
                with tc.tile_pool(name="p6", bufs=1) as p6, \
                     tc.tile_pool(name="pT", bufs=1) as pT:
                    # ---- LN2-local params + h2T affines (split across
                    # DVE and GpSimd: the chunk-0 affine chain is the only
                    # thing between phase 4 and the first FFN matmul) ----
                    def ln2_block(j):
                        sl = slice(j * CH, (j + 1) * CH)
                        loc_j = bn1_io[j][0]
                        A, Bt = norm_params_chunk(
                            pT, loc_j[:, 0:CH], loc_j[:, CH:2 * CH], C,
                            f"ln2_{j}", sc_tag="nsc", bi_tag="nbi",
                            bc_pool=h2pool, bc_bufs=2)
                        for k in range(CT):
                            affine_chunk(h2T[:, k, sl], u1[k][:, sl],
                                         A[:], Bt[:],
                                         eng=(nc.vector if k < 8
                                              else nc.gpsimd))

                    def bn1_block(j):
                        sl = slice(j * CH, (j + 1) * CH)
                        glob_j = bn1_io[j][1]
                        g1r = pT.tile([1, CH], fp32, tag="rg", bufs=1,
                                      name=f"bn1g_{j}")
                        nc.sync.dma_start(g1r[:], bn1g_d[0:1, sl])
                        b1r = pT.tile([1, CH], fp32, tag="rgb", bufs=1,
                                      name=f"bn1b_{j}")
                        nc.sync.dma_start(b1r[:], bn1b_d[0:1, sl])
                        sc, bi = norm_params_chunk(
                            pT, glob_j[:, 0:CH], glob_j[:, CH:2 * CH],
                            NBC, f"bn1_{j}", g_row_sl=g1r[:],
                            b_row_sl=b1r[:], sc_tag="bnsc",
                            bi_tag="bnbi", bc_pool=h2pool, bc_bufs=1,
                            via_dma=True)
                        for k in range(CT):
                            affine_chunk(u1[k][:, sl], u1[k][:, sl],
                                         sc[:], bi[:])

                    ln2_block(0)
                    bn1_block(0)

                    def bn2_params(j, via_dma=True):
                        sl = slice(j * CH, (j + 1) * CH)
                        g2r = pT.tile([1, CH], fp32, tag="rg", bufs=1,
                                      name=f"bn2g_{j}")
                        nc.sync.dma_start(g2r[:], bn2g_d[0:1, sl])
                        b2r = pT.tile([1, CH], fp32, tag="rgb", bufs=1,
                                      name=f"bn2b_{j}")
                        nc.sync.dma_start(b2r[:], bn2b_d[0:1, sl])
                        return norm_params_chunk(
                            pT, bn2_io[j][1][:, 0:CH],
                            bn2_io[j][1][:, CH:2 * CH],
                            NBC, f"bn2_{j}", g_row_sl=g2r[:],
                            b_row_sl=b2r[:], sc_tag="nsc", bi_tag="nbi",
                            bc_pool=h2pool, bc_bufs=2, via_dma=via_dma)

                    def bn2_finale_k(j, k, sc2, bi2, eng=None, q=None):
                        sl = slice(j * CH, (j + 1) * CH)
                        e = eng if eng is not None else nc.vector
                        qe = q if q is not None else nc.sync
                        tag = "ntmpg" if e is nc.gpsimd else "yout"
                        yk = (spool if e is nc.gpsimd else pT).tile(
                            [P, CH], fp32, tag=tag,
                            bufs=(1 if e is nc.gpsimd else 2),
                            name=f"y_{k}_{j}")
                        e.tensor_mul(yk[:], u1[k][:, sl], sc2[:])
                        e.tensor_add(yk[:], yk[:], bi2[:])
                        qe.dma_start(yT_d[k][:, sl], yk[:])

                    # ---- FFN, one chunk at a time (mm1 then mm2) ----
                    pp = None
                    for j in range(NCH):
                        sl = slice(j * CH, (j + 1) * CH)
                        z = []
                        for f in range(FT):
                            w1f = wstream.tile([P, CT, P], bf16, tag="w1f",
                                               bufs=2, name=f"w1f_{j}_{f}")
                            nc.scalar.dma_start(w1f[:], w1_d[f])
                            zps = ppw.tile([P, CH], fp32, tag="w",
                                           name=f"z_ps_{j}_{f}")
                            for k in range(CT):
                                nc.tensor.matmul(zps[:], w1f[:, k, :],
                                                 h2T[:, k, sl],
                                                 start=(k == 0),
                                                 stop=(k == CT - 1))
                            zf = p6.tile([P, CH], bf16, tag=f"z{f}",
                                         name=f"z_{j}_{f}")
                            nc.scalar.activation(zf[:], zps[:], AF.Relu,
                                                 bias=b1_sb[:, f:f + 1],
                                                 scale=1.0)
                            z.append(zf)
                        if j == 0:
                            ln2_block(1)
                            bn1_block(1)
                        if j > 0:
                            pp = bn2_params(j - 1)
                        for k in range(CT):
                            w2k = p6.tile([P, FT, P], bf16, tag="w2k", bufs=2,
                                          name=f"w2k_{j}_{k}")
                            nc.sync.dma_start(w2k[:], w2_d[k])
                            yps = ppw.tile([P, CH], fp32, tag="w",
                                           name=f"y_ps_{j}_{k}")
                            for f in range(FT):
                                nc.tensor.matmul(yps[:], w2k[:, f, :],
                                                 z[f][:],
                                                 start=(f == 0),
                                                 stop=(f == FT - 1))
                            nc.vector.scalar_tensor_tensor(
                                out=u1[k][:, sl], in0=yps[:],
                                scalar=b2_sb[:, k:k + 1], in1=u1[k][:, sl],
                                op0=OP.add, op1=OP.add)
                            stats_chunk(u1[k][:, sl], stp_bn2[j],
                                        k == 0, k == CT - 1)
                            if j > 0:
                                bn2_finale_k(j - 1, k, pp[0], pp[1])
                        bn2_io[j] = allreduce_chunk(u1pool, stp_bn2[j],
                                                    f"bn2_{j}")
                    # last chunk: PE is idle, use the low-latency matmul
                    # broadcast instead of the DMA round trip
                    sc2, bi2 = bn2_params(NCH - 1, via_dma=False)
                    for k in range(CT):
                        bn2_finale_k(NCH - 1, k, sc2, bi2,
                                     eng=(nc.vector if k < 8 else nc.gpsimd),
                                     q=(nc.sync if k % 2 == 0 else nc.scalar))

    nc.compile()
    return nc


def _get_program():
    global _PROG
    if _PROG is None:
        _PROG = _build()
    return _PROG


def _prep_shared(inputs):
    """Host-side weight folding + pre-tiling; identical for every core."""
    f32 = np.float32
    f8 = ml_dtypes.float8_e4m3fn
    wq = np.asarray(inputs["wq"], f32)      # (H, C, D)
    wk = np.asarray(inputs["wk"], f32)
    wv = np.asarray(inputs["wv"], f32)
    wo = np.asarray(inputs["wo"], f32)      # (C, C)
    bo = np.asarray(inputs["bo"], f32)      # (C,)
    g1 = np.asarray(inputs["ln1_g"], f32)
    b1n = np.asarray(inputs["ln1_b"], f32)
    g2 = np.asarray(inputs["ln2_g"], f32)
    b2n = np.asarray(inputs["ln2_b"], f32)
    w1 = np.asarray(inputs["w1"], f32)      # (C, F)
    b1 = np.asarray(inputs["b1"], f32)      # (F,)
    w2 = np.asarray(inputs["w2"], f32)      # (F, C)
    b2 = np.asarray(inputs["b2"], f32)      # (C,)

    dscale = f32(D) ** f32(-0.5)
    # fold ln1 affine into qkv projections; q also takes 1/sqrt(D)
    wq2 = (wq * g1[None, :, None] * dscale).transpose(1, 0, 2).reshape(C, C)
    wk2 = (wk * g1[None, :, None]).transpose(1, 0, 2).reshape(C, C)
    wv2 = (wv * g1[None, :, None]).transpose(1, 0, 2).reshape(C, C)
    bq = (np.einsum("c,hcd->hd", b1n, wq) * dscale).reshape(C)
    bv = np.einsum("c,hcd->hd", b1n, wv).reshape(C)
    # k-side bias cancels in softmax (constant per row); v bias folds into bo
    bo2 = bo + bv @ wo
    w1f = g2[:, None] * w1
    b1f = b1 + b2n @ w1

    def lhst_tiles(w, n_out, scale):
        # (C_in, n_out*P) -> (n_out, P, C_in//P, P):
        # [o, p, ki, n] = w[ki*P + p, o*P + n]
        ci = w.shape[0]
        return np.ascontiguousarray(
            (w * scale).reshape(ci // P, P, n_out, P).transpose(2, 1, 0, 3)
        ).astype(f8)

    def cols(v, n):  # (n*P,) -> (P, n) with [p, i] = v[i*P + p]
        return np.ascontiguousarray(v.reshape(n, P).T, dtype=f32)

    def row(v):
        return np.ascontiguousarray(v.reshape(1, T), dtype=f32)

    wv_t = wv2.astype(ml_dtypes.bfloat16)
    # wo stationary per k_out: [k, p, h, m] = wo[h*P + p, k*P + m]
    wo_t = np.ascontiguousarray(
        wo.reshape(H, P, CT, P).transpose(2, 1, 0, 3)).astype(ml_dtypes.bfloat16)

    def lhst_tiles_bf(w, n_out):
        ci = w.shape[0]
        return np.ascontiguousarray(
            w.reshape(ci // P, P, n_out, P).transpose(2, 1, 0, 3)
        ).astype(ml_dtypes.bfloat16)

    return dict(
        wq=lhst_tiles(wq2, H, SQ),
        wk=lhst_tiles(wk2, H, SK),
        wv=wv_t,
        bq=cols(bq, H), wo=wo_t,
        w1=lhst_tiles_bf(w1f, FT), b1=cols(b1f, FT),
        w2=lhst_tiles_bf(w2, CT), b2=cols(b2, CT),
        bn1g=row(np.asarray(inputs["bn1_g"], f32)),
        bn1b=row(np.asarray(inputs["bn1_b"], f32)),
        bn2g=row(np.asarray(inputs["bn2_g"], f32)),
        bn2b=row(np.asarray(inputs["bn2_b"], f32)),
        _bo2=bo2,
    )


def _run(inputs, trace=False):
    from concourse import bass_utils
    nc = _get_program()
    x = np.asarray(inputs["x"], np.float32)
    shared = _prep_shared(inputs)
    bo2 = shared.pop("_bo2")
    in_maps = []
    for b in range(B):
        m = dict(shared)
        xt = np.ascontiguousarray(x[b].T)                 # (C, T)
        m["xT"] = (xt + bo2[:, None]).reshape(CT, P, T)
        m["xbf"] = xt.astype(ml_dtypes.bfloat16).reshape(CT, P, T)
        in_maps.append(m)
    res = bass_utils.run_bass_kernel_spmd(
        nc, in_maps, core_ids=list(range(NCORES)), trace=trace)
    out = np.stack([
        res.results[b]["yT"].reshape(C, T).T for b in range(B)
    ]).astype(np.float32)
    return out, res


def kernel(**inputs):
    out, _ = _run(inputs, trace=False)
    return out
